# revision 3
# baseline (speedup 1.0000x reference)
"""Depthwise causal Conv1d (k=4) + SiLU on 8 Trainium2 NeuronCores.

Problem: x [4, 4096, 2048] f32, w [2048, 4] f32,
out[b, t, d] = silu(sum_j w[d, j] * x[b, t - 3 + j, d])   (zero-padded left).

Sharding: 8 cores = 4 batches x 2 channel-halves. Depthwise conv is
independent per channel, so channel sharding needs no halo exchange.

Layout: each core receives its shard host-transposed to [channels, time]
(channels on SBUF partitions). The per-channel weight w[d, j] is then a
per-partition scalar, and the causal time shifts are free-dim AP offsets
into one loaded tile.

Precision: x and the output are host-cast fp16 (halves HBM traffic both
ways); the MAC chain stays fp16 (PE accumulates fp32 in PSUM); SiLU
computes fp32-internally on ACT. End-to-end relative error ~5e-4.

v2 design (DMA-envelope bound, ~435 GB/s/core R+W combined):
 - Work is cut into [128ch, 2048t] chunks, interleaved across the 8
   channel blocks so loads, compute, and stores pipeline continuously
   (v1 stored nothing until 23us in; v2's first store issues ~4us).
 - 4 blocks ride the TensorEngine: diag(w_j) matmuls accumulate the 4
   taps in PSUM (1024-col quarters, 4 psum bufs), SiLU on ACT straight
   out of PSUM. The diag stationaries are built ON DEVICE from a [128,
   128] identity mask (tiny DMA) x per-partition weight scalars --
   v1 shipped a 1 MB diag tensor from HBM instead (~3us of DMA).
 - 4 blocks ride DVE as a fused MAC chain: q = x0*w0 then 3x
   scalar_tensor_tensor q = (xj*wj) + q. 4 ops/block vs v1's 7
   (products + add tree), and ACT no longer does any products.
 - All SiLU on ACT (it is the only engine with the table); loads issue
   on SyncE (HWDGE), stores on GpSimd (SWDGE).
First/last chunks are split finer (1024) to shorten pipeline ramp and
drain. Engine budgets per core: DMA ~40us (16.8 MB), PE ~34us,
DVE ~34us, ACT ~33us.
"""

import sys
import types

import numpy as np

import concourse.bass as bass
import concourse.bacc as bacc
import concourse.mybir as mybir
from concourse.tile import TileContext
from concourse.bass_utils import run_bass_kernel_spmd


def _ensure_ntff_hook():
    """bass_utils imports antenv.axon_hooks when BASS_TRACE is set; that
    module is absent on this image. Install a shim so tracing works when
    possible and degrades gracefully (instead of crashing) when not."""
    try:
        import antenv.axon_hooks  # noqa: F401

        return
    except ImportError:
        pass
    try:
        import antenv

        hook = None
        try:
            if "/root/.axon_site" not in sys.path:
                sys.path.insert(0, "/root/.axon_site")
            from trn_agent_boot.trn_boot import _ntff_profile_via_ctypes

            hook = _ntff_profile_via_ctypes("/opt/axon/libaxon_pjrt.so")
        except Exception:
            hook = None
        mod = types.ModuleType("antenv.axon_hooks")
        mod._hook = hook
        mod.get_axon_ntff_profile_hook = lambda: mod._hook
        mod.set_axon_ntff_profile_hook = lambda h: setattr(mod, "_hook", h)
        sys.modules["antenv.axon_hooks"] = mod
        antenv.axon_hooks = mod
    except Exception:
        pass


_ensure_ntff_hook()

B, L, D = 4, 4096, 2048
K = 4
PAD = K - 1
N_CORES = 8
DH = D // 2            # channels per core
NBLK = DH // 128       # 128-partition channel blocks per core
ROWW = 4128            # DRAM row stride (fp16 elems): 64B-aligned rows

MID_DT = mybir.dt.float16
PE_BLKS = [1, 3, 5, 7]      # blocks on the TensorEngine (interleaved with DVE)
_PE_IDX = {b: i for i, b in enumerate(PE_BLKS)}

_cache = {}


def _build_bass():
    nc = bacc.Bacc()
    xt = nc.dram_tensor("xt", [DH, ROWW], MID_DT, kind="ExternalInput")
    wt = nc.dram_tensor("wt", [128, NBLK * K], mybir.dt.float32, kind="ExternalInput")
    # [128,128] identity mask; diag(w_j) stationaries are built on device
    dg = nc.dram_tensor("dg", [128, 128], MID_DT, kind="ExternalInput")
    ot = nc.dram_tensor("ot", [DH, L], MID_DT, kind="ExternalOutput")
    f32 = mybir.dt.float32
    mult = mybir.AluOpType.mult
    add = mybir.AluOpType.add

    with TileContext(nc) as tc:
        with tc.tile_pool(name="pool", bufs=2) as pool, \
             tc.tile_pool(name="psum", bufs=2, space="PSUM") as psum_pool:
            # Warmup: a tiny Silu forces the silu activation-table set to
            # load during the initial DMA wait; it is the only table load
            # in the whole kernel.
            warm = pool.tile([128, 2], MID_DT, tag="warm", bufs=1)
            nc.vector.memset(warm[:], 0.0)
            nc.scalar.activation(warm[:], warm[:], mybir.ActivationFunctionType.Silu)

            w = pool.tile([128, NBLK * K], f32, tag="w", bufs=1)
            nc.sync.dma_start(out=w[:], in_=wt[:, :])
            mask = pool.tile([128, 128], MID_DT, tag="mask", bufs=1)
            nc.sync.dma_start(out=mask[:], in_=dg[:, :])

            # diag(w[blk*128+p, j]) stationaries for the PE path: 16 cheap
            # [128,128] per-partition-scalar muls of the identity mask.
            dgw = pool.tile([128, len(PE_BLKS) * K * 128], MID_DT, tag="dgw", bufs=1)
            for blk in PE_BLKS:
                bi = _PE_IDX[blk]
                for j in range(K):
                    c0 = (bi * K + j) * 128
                    nc.vector.tensor_scalar_mul(
                        dgw[:, c0 : c0 + 128],
                        mask[:],
                        w[:, blk * K + j : blk * K + j + 1],
                    )

            def wj(blk, j):
                return w[:, blk * K + j : blk * K + j + 1]

            def pe_chunk(blk, t0, tl):
                """TensorEngine path: accumulate the 4 taps as diag(w_j)
                matmuls in 1024-col PSUM quarters, SiLU from PSUM."""
                r0 = blk * 128
                bi = _PE_IDX[blk]
                x = pool.tile([128, tl + PAD + 1], MID_DT, tag="x", bufs=6)
                nc.sync.dma_start(
                    out=x[:, 0 : tl + PAD], in_=xt[r0 : r0 + 128, t0 : t0 + tl + PAD]
                )
                o = pool.tile([128, tl], MID_DT, tag="o", bufs=6)
                for h0 in range(0, tl, 1024):
                    pq = min(1024, tl - h0)
                    ps = psum_pool.tile([128, pq], f32, tag="ps", bufs=4)
                    for c in range(pq // 512):
                        for j in range(K):
                            lw = dgw[:, (bi * K + j) * 128 : (bi * K + j + 1) * 128]
                            nc.tensor.matmul(
                                ps[:, c * 512 : (c + 1) * 512],
                                lw,
                                x[:, h0 + c * 512 + j : h0 + c * 512 + j + 512],
                                start=(j == 0),
                                stop=(j == K - 1),
                            )
                    nc.scalar.activation(
                        o[:, h0 : h0 + pq], ps[:], mybir.ActivationFunctionType.Silu
                    )
                nc.gpsimd.dma_start(out=ot[r0 : r0 + 128, t0 : t0 + tl], in_=o[:])

            def dve_chunk(blk, t0, tl):
                """DVE path: fused MAC chain, q = sum_j w_j * x[t+j]."""
                r0 = blk * 128
                x = pool.tile([128, tl + PAD + 1], MID_DT, tag="x", bufs=6)
                nc.sync.dma_start(
                    out=x[:, 0 : tl + PAD], in_=xt[r0 : r0 + 128, t0 : t0 + tl + PAD]
                )
                q = pool.tile([128, tl], MID_DT, tag="q", bufs=3)
                nc.vector.tensor_scalar_mul(q[:], x[:, 0:tl], wj(blk, 0))
                for j in range(1, K):
                    nc.vector.scalar_tensor_tensor(
                        q[:], x[:, j : j + tl], wj(blk, j), q[:], mult, add
                    )
                o = pool.tile([128, tl], MID_DT, tag="o", bufs=6)
                nc.scalar.activation(o[:], q[:], mybir.ActivationFunctionType.Silu)
                nc.gpsimd.dma_start(out=ot[r0 : r0 + 128, t0 : t0 + tl], in_=o[:])

            def chunk(blk, t0, tl):
                if blk in _PE_IDX:
                    pe_chunk(blk, t0, tl)
                else:
                    dve_chunk(blk, t0, tl)

            C = 2048
            # Half 0 (t in [0, 2048)): first block split fine so the first
            # store issues early; E/P blocks interleave for engine balance.
            chunk(0, 0, 1024)
            chunk(0, 1024, 1024)
            for blk in [1, 2, 3, 4, 5, 6, 7]:
                chunk(blk, 0, C)
            # Half 1: heavy PE chunks first, last E block split fine so the
            # pipeline drains fast after the final load.
            for blk in [1, 0, 3, 2, 5, 4, 7]:
                chunk(blk, C, C)
            chunk(6, C, 1024)
            chunk(6, C + 1024, 1024)
    nc.compile()
    return nc


def _shard_inputs(x, w):
    in_maps = []
    dg = np.eye(128, dtype=np.float16)
    for core in range(N_CORES):
        b, half = divmod(core, 2)
        d0 = half * DH
        xt = np.zeros((DH, ROWW), dtype=np.float16)
        xt[:, PAD : PAD + L] = x[b, :, d0 : d0 + DH].T.astype(np.float16)
        # w rows for this shard, rearranged so partition p holds the K
        # weights of channel blk*128 + p at free cols [blk*K, blk*K + K)
        w_sh = w[d0 : d0 + DH].reshape(NBLK, 128, K)
        wt = (
            w_sh.transpose(1, 0, 2).reshape(128, NBLK * K).astype(np.float32)
        )
        in_maps.append(
            {
                "xt": np.ascontiguousarray(xt),
                "wt": np.ascontiguousarray(wt),
                "dg": dg,
            }
        )
    return in_maps


def kernel(x, w):
    x = np.asarray(x, dtype=np.float32)
    w = np.asarray(w, dtype=np.float32)
    assert x.shape == (B, L, D) and w.shape == (D, K)

    if "nc" not in _cache:
        _cache["nc"] = _build_bass()
    nc = _cache["nc"]

    in_maps = _shard_inputs(x, w)
    res = None
    for attempt in range(3):
        try:
            res = run_bass_kernel_spmd(nc, in_maps, core_ids=list(range(N_CORES)))
            break
        except Exception:
            if attempt == 2:
                raise
    _cache["last_results"] = res

    out = np.empty((B, L, D), dtype=np.float32)
    for core in range(N_CORES):
        b, half = divmod(core, 2)
        d0 = half * DH
        out[b, :, d0 : d0 + DH] = res.results[core]["ot"].T.astype(np.float32)
    return out


# revision 4
# speedup vs baseline: 1.2648x; 1.2648x over previous
"""Depthwise causal Conv1d (k=4) + SiLU on 8 Trainium2 NeuronCores.

Problem: x [4, 4096, 2048] f32, w [2048, 4] f32,
out[b, t, d] = silu(sum_j w[d, j] * x[b, t - 3 + j, d])   (zero-padded left).

Sharding: 8 cores = 4 batches x 2 channel-halves. Depthwise conv is
independent per channel, so channel sharding needs no halo exchange.

Layout: each core receives its shard host-transposed to [channels, time]
(channels on SBUF partitions). The per-channel weight w[d, j] is then a
per-partition scalar, and the causal time shifts are free-dim AP offsets
into one loaded tile.

Precision: x and the output are host-cast fp16 (halves HBM traffic both
ways); products and adds stay fp16 (PE accumulates fp32 in PSUM); SiLU
computes fp32-internally on ACT. End-to-end relative error ~5e-4.

v3 design, tuned against the NTFF profile of v1/v2:
 - The per-core budget is DMA: 16.8 MB at ~435 GB/s (R+W combined)
   = ~40us. Every engine is kept below that envelope.
 - The TensorEngine has p-states (0.65/1.2/2.4 GHz) and only reaches
   2.4 GHz after ~3us of CONTINUOUS execution, so ALL PE work (5 of 8
   channel blocks as diag(w_j) matmuls, 4 taps PSUM-accumulated) is one
   back-to-back stream of 10 [128,2048] chunks -> ~34us at full clock.
   Diag stationaries are built on device from a [128,128] identity mask
   (v1 shipped them as a 1 MB HBM tensor instead).
 - 3 blocks ride DVE with 4 tensor_scalar products (shift-rebased) and
   pair-packed adds (~6us per chunk; scalar_tensor_tensor MAC chains
   measured 2x slower per column -- no DVE fast path -- so v2's fused
   chain was reverted).
 - ACT does only SiLU (v1 gave it products too): PE chunks straight out
   of PSUM as one [128,2048] op, DVE chunks from SBUF.
 - Work is emitted chunk-interleaved (P:E ~ 10:6) so loads, compute and
   stores pipeline; first/last chunks split at 1024 to shorten ramp and
   drain. Loads issue on SyncE (HWDGE, bufs=8 runway), stores on GpSimd
   (SWDGE).
"""

import sys
import types

import numpy as np

import concourse.bass as bass
import concourse.bacc as bacc
import concourse.mybir as mybir
from concourse.tile import TileContext
from concourse.bass_utils import run_bass_kernel_spmd


def _ensure_ntff_hook():
    """bass_utils imports antenv.axon_hooks when BASS_TRACE is set; that
    module is absent on this image. Install a shim so tracing works when
    possible and degrades gracefully (instead of crashing) when not."""
    try:
        import antenv.axon_hooks  # noqa: F401

        return
    except ImportError:
        pass
    try:
        import antenv

        hook = None
        try:
            if "/root/.axon_site" not in sys.path:
                sys.path.insert(0, "/root/.axon_site")
            from trn_agent_boot.trn_boot import _ntff_profile_via_ctypes

            hook = _ntff_profile_via_ctypes("/opt/axon/libaxon_pjrt.so")
        except Exception:
            hook = None
        mod = types.ModuleType("antenv.axon_hooks")
        mod._hook = hook
        mod.get_axon_ntff_profile_hook = lambda: mod._hook
        mod.set_axon_ntff_profile_hook = lambda h: setattr(mod, "_hook", h)
        sys.modules["antenv.axon_hooks"] = mod
        antenv.axon_hooks = mod
    except Exception:
        pass


_ensure_ntff_hook()

B, L, D = 4, 4096, 2048
K = 4
PAD = K - 1
N_CORES = 8
DH = D // 2            # channels per core
NBLK = DH // 128       # 128-partition channel blocks per core
ROWW = 4128            # DRAM row stride (fp16 elems): 64B-aligned rows

MID_DT = mybir.dt.float16
PE_BLKS = [1, 3, 5, 7, 6]   # blocks on the TensorEngine
DVE_BLKS = [0, 2, 4]        # blocks on DVE
_PE_IDX = {b: i for i, b in enumerate(PE_BLKS)}
C = 2048

_cache = {}


def _build_bass():
    nc = bacc.Bacc()
    xt = nc.dram_tensor("xt", [DH, ROWW], MID_DT, kind="ExternalInput")
    wt = nc.dram_tensor("wt", [128, NBLK * K], mybir.dt.float32, kind="ExternalInput")
    # [128,128] identity mask; diag(w_j) stationaries are built on device
    dg = nc.dram_tensor("dg", [128, 128], MID_DT, kind="ExternalInput")
    ot = nc.dram_tensor("ot", [DH, L], MID_DT, kind="ExternalOutput")
    f32 = mybir.dt.float32

    with TileContext(nc) as tc:
        with tc.tile_pool(name="pool", bufs=2) as pool, \
             tc.tile_pool(name="psum", bufs=2, space="PSUM") as psum_pool:
            # Warmup: a tiny Silu forces the silu activation-table set to
            # load during the initial DMA wait; it is the only table load
            # in the whole kernel.
            warm = pool.tile([128, 2], MID_DT, tag="warm", bufs=1)
            nc.vector.memset(warm[:], 0.0)
            nc.scalar.activation(warm[:], warm[:], mybir.ActivationFunctionType.Silu)

            w = pool.tile([128, NBLK * K], f32, tag="w", bufs=1)
            nc.sync.dma_start(out=w[:], in_=wt[:, :])
            mask = pool.tile([128, 128], MID_DT, tag="mask", bufs=1)
            nc.sync.dma_start(out=mask[:], in_=dg[:, :])

            # diag(w[blk*128+p, j]) stationaries for the PE path: cheap
            # [128,128] per-partition-scalar muls of the identity mask.
            dgw = pool.tile([128, len(PE_BLKS) * K * 128], MID_DT, tag="dgw", bufs=1)
            for blk in PE_BLKS:
                bi = _PE_IDX[blk]
                for j in range(K):
                    c0 = (bi * K + j) * 128
                    nc.vector.tensor_scalar_mul(
                        dgw[:, c0 : c0 + 128],
                        mask[:],
                        w[:, blk * K + j : blk * K + j + 1],
                    )

            def wj(blk, j):
                return w[:, blk * K + j : blk * K + j + 1]

            def pe_chunk(args):
                """TensorEngine path: the 4 taps accumulate as diag(w_j)
                matmuls in a [128, tl] PSUM tile, one SiLU from PSUM."""
                blk, t0, tl = args
                r0 = blk * 128
                bi = _PE_IDX[blk]
                x = pool.tile([128, tl + PAD + 1], MID_DT, tag="x", bufs=8)
                nc.sync.dma_start(
                    out=x[:, 0 : tl + PAD], in_=xt[r0 : r0 + 128, t0 : t0 + tl + PAD]
                )
                o = pool.tile([128, tl], MID_DT, tag="o", bufs=6)
                ps = psum_pool.tile([128, tl], f32, tag="ps", bufs=2)
                for c in range(tl // 512):
                    for j in range(K):
                        lw = dgw[:, (bi * K + j) * 128 : (bi * K + j + 1) * 128]
                        nc.tensor.matmul(
                            ps[:, c * 512 : (c + 1) * 512],
                            lw,
                            x[:, c * 512 + j : c * 512 + j + 512],
                            start=(j == 0),
                            stop=(j == K - 1),
                        )
                nc.scalar.activation(
                    o[:], ps[:], mybir.ActivationFunctionType.Silu
                )
                nc.gpsimd.dma_start(out=ot[r0 : r0 + 128, t0 : t0 + tl], in_=o[:])

            def dve_chunk(args):
                """DVE path: 4 shift-rebased tensor_scalar products, then
                pair-packed adds (qe=[q0|q2], qo=[q1|q3])."""
                blk, t0, tl = args
                r0 = blk * 128
                x = pool.tile([128, tl + PAD + 1], MID_DT, tag="x", bufs=8)
                nc.sync.dma_start(
                    out=x[:, 0 : tl + PAD], in_=xt[r0 : r0 + 128, t0 : t0 + tl + PAD]
                )
                qe = pool.tile([128, 2, tl], MID_DT, tag="qe", bufs=2)
                qo = pool.tile([128, 2, tl], MID_DT, tag="qo", bufs=2)
                nc.vector.tensor_scalar_mul(qe[:, 0, :], x[:, 0:tl], wj(blk, 0))
                nc.vector.tensor_scalar_mul(qo[:, 0, :], x[:, 1 : 1 + tl], wj(blk, 1))
                nc.vector.tensor_scalar_mul(qe[:, 1, :], x[:, 2 : 2 + tl], wj(blk, 2))
                nc.vector.tensor_scalar_mul(qo[:, 1, :], x[:, 3 : 3 + tl], wj(blk, 3))
                nc.vector.tensor_add(qe[:, :, :], qe[:, :, :], qo[:, :, :])
                nc.vector.tensor_add(qe[:, 0, :], qe[:, 0, :], qe[:, 1, :])
                o = pool.tile([128, tl], MID_DT, tag="o", bufs=6)
                nc.scalar.activation(
                    o[:], qe[:, 0, :], mybir.ActivationFunctionType.Silu
                )
                nc.gpsimd.dma_start(out=ot[r0 : r0 + 128, t0 : t0 + tl], in_=o[:])

            # PE chunks run back-to-back (p-state ramp); DVE chunks fill in.
            # First/last DVE chunks split at 1024 for faster ramp/drain.
            P = [(b, t0, C) for t0 in (0, C) for b in PE_BLKS]
            E = ([(0, 0, 1024), (0, 1024, 1024), (2, 0, C), (4, 0, C),
                  (0, C, C), (2, C, C), (4, C, 1024), (4, C + 1024, 1024)])
            order = [E[0], P[0], E[1], P[1], P[2], E[2], P[3], P[4], E[3],
                     P[5], P[6], E[4], P[7], P[8], E[5], P[9], E[6], E[7]]
            for item in order:
                if item[0] in _PE_IDX:
                    pe_chunk(item)
                else:
                    dve_chunk(item)
    nc.compile()
    return nc


def _shard_inputs(x, w):
    in_maps = []
    dg = np.eye(128, dtype=np.float16)
    for core in range(N_CORES):
        b, half = divmod(core, 2)
        d0 = half * DH
        xt = np.zeros((DH, ROWW), dtype=np.float16)
        xt[:, PAD : PAD + L] = x[b, :, d0 : d0 + DH].T.astype(np.float16)
        # w rows for this shard, rearranged so partition p holds the K
        # weights of channel blk*128 + p at free cols [blk*K, blk*K + K)
        w_sh = w[d0 : d0 + DH].reshape(NBLK, 128, K)
        wt = (
            w_sh.transpose(1, 0, 2).reshape(128, NBLK * K).astype(np.float32)
        )
        in_maps.append(
            {
                "xt": np.ascontiguousarray(xt),
                "wt": np.ascontiguousarray(wt),
                "dg": dg,
            }
        )
    return in_maps


def kernel(x, w):
    x = np.asarray(x, dtype=np.float32)
    w = np.asarray(w, dtype=np.float32)
    assert x.shape == (B, L, D) and w.shape == (D, K)

    if "nc" not in _cache:
        _cache["nc"] = _build_bass()
    nc = _cache["nc"]

    in_maps = _shard_inputs(x, w)
    res = None
    for attempt in range(3):
        try:
            res = run_bass_kernel_spmd(nc, in_maps, core_ids=list(range(N_CORES)))
            break
        except Exception:
            if attempt == 2:
                raise
    _cache["last_results"] = res

    out = np.empty((B, L, D), dtype=np.float32)
    for core in range(N_CORES):
        b, half = divmod(core, 2)
        d0 = half * DH
        out[b, :, d0 : d0 + DH] = res.results[core]["ot"].T.astype(np.float32)
    return out
